# revision 1
# baseline (speedup 1.0000x reference)
"""Trainium2 Bass kernel for nn_EnsembleModel (ensemble MLP, E=10).

Computes, for each ensemble member e:
    h1 = silu(x @ W1[e] + b1[e])      # [B, 256]
    h2 = silu(h1 @ W2[e] + b2[e])     # [B, 256]
    pred = h2 @ W3[e] + b3[e]         # [B, 48]
    means, logvars = pred[:, :24], pred[:, 24:]

Sharding: data-parallel over the batch axis across 8 NeuronCores; all
ensemble weights are replicated and SBUF-resident on every core.

On-chip layout: features live on SBUF partitions, batch on the free dim.
Matmuls are W.T @ x with the weight stationary ([K, M]) and the batch
streaming as the moving operand (N=512 per PSUM bank).  Silu+bias is fused
into one ScalarE activation per [128, 1024] tile reading PSUM directly.
"""

import numpy as np
import ml_dtypes

import concourse.bacc as bacc
import concourse.mybir as mybir
import concourse.tile as tile
from concourse.bass_utils import run_bass_kernel_spmd

# problem dims (hardcoded per harness contract)
E = 10          # ensemble members
IN = 32         # input dim
H = 256         # hidden
OUT = 24        # output dim (head emits 2*OUT)
OUT2 = 2 * OUT  # 48
B = 65536
NCORES = 8
BL = B // NCORES    # 8192 batch rows per core
BT = 512            # matmul moving free dim (one fp32 PSUM bank)
GRP = 2             # batch tiles per iteration group
GW = BT * GRP       # 1024
NGRP = BL // GW     # 8

F32 = mybir.dt.float32

# compute dtype for matmul operands ("bf16" or "f32")
COMPUTE_DTYPE = "bf16"


def build_nc(compute="bf16", inner_reps=1):
    """Build the per-core Bass program (SPMD: same NEFF on all 8 cores)."""
    dt = mybir.dt.bfloat16 if compute == "bf16" else mybir.dt.float32
    nc = bacc.Bacc("TRN2", target_bir_lowering=False, debug=False)

    xT_d = nc.dram_tensor("xT", [IN, BL], dt, kind="ExternalInput")
    w1_d = nc.dram_tensor("w1", [IN, E * H], dt, kind="ExternalInput")
    w2_d = nc.dram_tensor("w2", [128, E * 2 * H], dt, kind="ExternalInput")
    w3_d = nc.dram_tensor("w3", [128, E * 2 * OUT2], dt, kind="ExternalInput")
    bs_d = nc.dram_tensor("bs", [128, 5 * E], F32, kind="ExternalInput")
    out_d = nc.dram_tensor("out", [E, OUT2, BL], F32, kind="ExternalOutput")

    xT = xT_d.ap()
    w1 = w1_d.ap()
    w2 = w2_d.ap()
    w3 = w3_d.ap()
    bs = bs_d.ap()
    out = out_d.ap()

    Silu = mybir.ActivationFunctionType.Silu

    with tile.TileContext(nc) as tc:
        with (
            tc.tile_pool(name="consts", bufs=1) as cpool,
            tc.tile_pool(name="hsb", bufs=6) as hpool,
            tc.tile_pool(name="osb", bufs=2) as opool,
            tc.tile_pool(name="hps", bufs=3, space="PSUM") as hps,
            tc.tile_pool(name="ops", bufs=1, space="PSUM") as ops,
        ):
            xsb = cpool.tile([IN, BL], dt, name="xsb")
            nc.sync.dma_start(xsb[:], xT)
            w1sb = cpool.tile([IN, E * H], dt, name="w1sb")
            nc.sync.dma_start(w1sb[:], w1)
            w2sb = cpool.tile([128, E * 2 * H], dt, name="w2sb")
            nc.sync.dma_start(w2sb[:], w2)
            w3sb = cpool.tile([128, E * 2 * OUT2], dt, name="w3sb")
            nc.sync.dma_start(w3sb[:], w3)
            bsb = cpool.tile([128, 5 * E], F32, name="bsb")
            nc.sync.dma_start(bsb[:], bs)

            for rep in range(inner_reps):
                for g in range(NGRP):
                    c0 = g * GW
                    for e in range(E):
                        uid = f"r{rep}g{g}e{e}"
                        # ---- layer 1: z1 = W1[e].T @ x  (K=32, M=256) ----
                        h1ps = [
                            hps.tile([128, GW], F32, tag="h", name=f"h1ps{uid}m{m}")
                            for m in range(2)
                        ]
                        for m in range(2):
                            lhsT = w1sb[:, e * H + m * 128 : e * H + (m + 1) * 128]
                            for t in range(GRP):
                                ts = slice(t * BT, (t + 1) * BT)
                                nc.tensor.matmul(
                                    h1ps[m][:, ts],
                                    lhsT,
                                    xsb[:, c0 + t * BT : c0 + (t + 1) * BT],
                                    start=True,
                                    stop=True,
                                )
                        # silu(z1 + b1) fused on ScalarE, PSUM -> SBUF(dt)
                        h1sb = [
                            hpool.tile([128, GW], dt, tag="hs", name=f"h1sb{uid}m{m}")
                            for m in range(2)
                        ]
                        for m in range(2):
                            nc.scalar.activation(
                                h1sb[m][:],
                                h1ps[m][:],
                                Silu,
                                bias=bsb[:, 2 * e + m : 2 * e + m + 1],
                            )

                        # ---- layer 2: z2 = W2[e].T @ h1 (K=256 via 2 k-tiles) ----
                        h2ps = [
                            hps.tile([128, GW], F32, tag="h", name=f"h2ps{uid}m{m}")
                            for m in range(2)
                        ]
                        for m in range(2):
                            for k in range(2):
                                base = (2 * e + k) * H + m * 128
                                lhsT = w2sb[:, base : base + 128]
                                for t in range(GRP):
                                    ts = slice(t * BT, (t + 1) * BT)
                                    nc.tensor.matmul(
                                        h2ps[m][:, ts],
                                        lhsT,
                                        h1sb[k][:, ts],
                                        start=(k == 0),
                                        stop=(k == 1),
                                    )
                        h2sb = [
                            hpool.tile([128, GW], dt, tag="hs", name=f"h2sb{uid}m{m}")
                            for m in range(2)
                        ]
                        for m in range(2):
                            nc.scalar.activation(
                                h2sb[m][:],
                                h2ps[m][:],
                                Silu,
                                bias=bsb[:, 2 * E + 2 * e + m : 2 * E + 2 * e + m + 1],
                            )

                        # ---- layer 3: pred = W3[e].T @ h2 (K=256, M=48) ----
                        pps = ops.tile([OUT2, GW], F32, tag="o", name=f"pps{uid}")
                        for k in range(2):
                            base = (2 * e + k) * OUT2
                            lhsT = w3sb[:, base : base + OUT2]
                            for t in range(GRP):
                                ts = slice(t * BT, (t + 1) * BT)
                                nc.tensor.matmul(
                                    pps[:, ts],
                                    lhsT,
                                    h2sb[k][:, ts],
                                    start=(k == 0),
                                    stop=(k == 1),
                                )
                        # bias add on VectorE (PSUM -> SBUF fp32), then store
                        osb = opool.tile([OUT2, GW], F32, tag="os", name=f"osb{uid}")
                        nc.vector.tensor_scalar_add(
                            osb[:], pps[:], bsb[:OUT2, 4 * E + e : 4 * E + e + 1]
                        )
                        nc.sync.dma_start(out[e, :, c0 : c0 + GW], osb[:])

    nc.finalize()
    return nc


def prep_inputs(x, W1, b1, W2, b2, W3, b3, compute="bf16"):
    """Host-side shard + repack into the per-core DRAM layouts."""
    npdt = ml_dtypes.bfloat16 if compute == "bf16" else np.float32
    x = np.asarray(x, np.float32)
    W1 = np.asarray(W1, np.float32)
    W2 = np.asarray(W2, np.float32)
    W3 = np.asarray(W3, np.float32)
    b1 = np.asarray(b1, np.float32)
    b2 = np.asarray(b2, np.float32)
    b3 = np.asarray(b3, np.float32)

    # weights: [K, M] stationary layouts, shared by all cores
    w1h = np.ascontiguousarray(W1.transpose(1, 0, 2).reshape(IN, E * H)).astype(npdt)
    w2h = np.ascontiguousarray(
        W2.reshape(E, 2, 128, H).transpose(2, 0, 1, 3).reshape(128, E * 2 * H)
    ).astype(npdt)
    w3h = np.ascontiguousarray(
        W3.reshape(E, 2, 128, OUT2).transpose(2, 0, 1, 3).reshape(128, E * 2 * OUT2)
    ).astype(npdt)

    # biases: [128, 5E] fp32; col 2e+m -> b1[e] m-half, col 2E+2e+m -> b2,
    # col 4E+e -> b3[e] in rows 0:48
    bsh = np.zeros((128, 5 * E), np.float32)
    for e in range(E):
        for m in range(2):
            bsh[:, 2 * e + m] = b1[e, m * 128 : (m + 1) * 128]
            bsh[:, 2 * E + 2 * e + m] = b2[e, m * 128 : (m + 1) * 128]
        bsh[:OUT2, 4 * E + e] = b3[e]

    in_maps = []
    xs = x.reshape(NCORES, BL, IN)
    for c in range(NCORES):
        xTc = np.ascontiguousarray(xs[c].T).astype(npdt)  # [IN, BL]
        in_maps.append(
            {"xT": xTc, "w1": w1h, "w2": w2h, "w3": w3h, "bs": bsh}
        )
    return in_maps


def assemble(results):
    """Per-core [E, 48, BL] fp32 -> (means, logvars) [E, B, 24] fp32."""
    full = np.concatenate([r["out"] for r in results], axis=2)  # [E, 48, B]
    pred = np.ascontiguousarray(full.transpose(0, 2, 1))        # [E, B, 48]
    means = np.ascontiguousarray(pred[:, :, :OUT])
    logvars = np.ascontiguousarray(pred[:, :, OUT:])
    return means, logvars


_NC_CACHE = {}


def _get_nc(compute, inner_reps=1):
    key = (compute, inner_reps)
    if key not in _NC_CACHE:
        _NC_CACHE[key] = build_nc(compute, inner_reps)
    return _NC_CACHE[key]


def kernel(x, W1, b1, W2, b2, W3, b3):
    compute = COMPUTE_DTYPE
    nc = _get_nc(compute)
    in_maps = prep_inputs(x, W1, b1, W2, b2, W3, b3, compute)
    res = run_bass_kernel_spmd(nc, in_maps, core_ids=list(range(NCORES)))
    return assemble(res.results)


# revision 3
# speedup vs baseline: 220.3682x; 220.3682x over previous
"""Trainium2 Bass kernel for nn_EnsembleModel (ensemble MLP, E=10).

Computes, for each ensemble member e:
    h1 = silu(x @ W1[e] + b1[e])      # [B, 256]
    h2 = silu(h1 @ W2[e] + b2[e])     # [B, 256]
    pred = h2 @ W3[e] + b3[e]         # [B, 48]
    means, logvars = pred[:, :24], pred[:, 24:]

Sharding: data-parallel over the batch axis across 8 NeuronCores; all
ensemble weights are replicated and SBUF-resident on every core.

On-chip layout: features live on SBUF partitions, batch on the free dim.
Matmuls are W.T @ x with the weight stationary ([K, M]) and the batch
streaming as the moving operand (N=512 per PSUM bank).  Silu+bias is fused
into one ScalarE activation per [128, 1024] tile reading PSUM directly.
"""

import numpy as np
import ml_dtypes

import concourse.bacc as bacc
import concourse.mybir as mybir
import concourse.tile as tile
from concourse.bass_utils import run_bass_kernel_spmd

# problem dims (hardcoded per harness contract)
E = 10          # ensemble members
IN = 32         # input dim
H = 256         # hidden
OUT = 24        # output dim (head emits 2*OUT)
OUT2 = 2 * OUT  # 48
B = 65536
NCORES = 8
BL = B // NCORES    # 8192 batch rows per core
BT = 512            # matmul moving free dim (one fp32 PSUM bank)
GRP = 2             # batch tiles per iteration group
GW = BT * GRP       # 1024
NGRP = BL // GW     # 8

F32 = mybir.dt.float32

# compute dtype for matmul operands ("bf16" or "f32")
COMPUTE_DTYPE = "bf16"


def build_nc(compute="bf16", inner_reps=1):
    """Build the per-core Bass program (SPMD: same NEFF on all 8 cores)."""
    dt = mybir.dt.bfloat16 if compute == "bf16" else mybir.dt.float32
    nc = bacc.Bacc("TRN2", target_bir_lowering=False, debug=False)

    xT_d = nc.dram_tensor("xT", [IN, BL], dt, kind="ExternalInput")
    w1_d = nc.dram_tensor("w1", [IN, E * H], dt, kind="ExternalInput")
    w2_d = nc.dram_tensor("w2", [128, E * 2 * H], dt, kind="ExternalInput")
    w3_d = nc.dram_tensor("w3", [128, E * 2 * OUT2], dt, kind="ExternalInput")
    bs_d = nc.dram_tensor("bs", [128, 5 * E], F32, kind="ExternalInput")
    out_d = nc.dram_tensor("out", [E, OUT2, BL], F32, kind="ExternalOutput")

    xT = xT_d.ap()
    w1 = w1_d.ap()
    w2 = w2_d.ap()
    w3 = w3_d.ap()
    bs = bs_d.ap()
    out = out_d.ap()

    Silu = mybir.ActivationFunctionType.Silu

    with tile.TileContext(nc) as tc:
        with (
            tc.tile_pool(name="consts", bufs=1) as cpool,
            tc.tile_pool(name="hsb", bufs=6) as hpool,
            tc.tile_pool(name="osb", bufs=2) as opool,
            tc.tile_pool(name="hps", bufs=3, space="PSUM") as hps,
            tc.tile_pool(name="ops", bufs=1, space="PSUM") as ops,
        ):
            xsb = cpool.tile([IN, BL], dt, name="xsb")
            nc.sync.dma_start(xsb[:], xT)
            w1sb = cpool.tile([IN, E * H], dt, name="w1sb")
            nc.sync.dma_start(w1sb[:], w1)
            w2sb = cpool.tile([128, E * 2 * H], dt, name="w2sb")
            nc.sync.dma_start(w2sb[:], w2)
            w3sb = cpool.tile([128, E * 2 * OUT2], dt, name="w3sb")
            nc.sync.dma_start(w3sb[:], w3)
            bsb = cpool.tile([128, 5 * E], F32, name="bsb")
            nc.sync.dma_start(bsb[:], bs)

            def body(rep):
                for g in range(NGRP):
                    c0 = g * GW
                    for e in range(E):
                        uid = f"r{rep}g{g}e{e}"
                        # ---- layer 1: z1 = W1[e].T @ x  (K=32, M=256) ----
                        h1ps = [
                            hps.tile([128, GW], F32, tag="h", name=f"h1ps{uid}m{m}")
                            for m in range(2)
                        ]
                        for m in range(2):
                            lhsT = w1sb[:, e * H + m * 128 : e * H + (m + 1) * 128]
                            for t in range(GRP):
                                ts = slice(t * BT, (t + 1) * BT)
                                nc.tensor.matmul(
                                    h1ps[m][:, ts],
                                    lhsT,
                                    xsb[:, c0 + t * BT : c0 + (t + 1) * BT],
                                    start=True,
                                    stop=True,
                                )
                        # silu(z1 + b1) fused on ScalarE, PSUM -> SBUF(dt)
                        h1sb = [
                            hpool.tile([128, GW], dt, tag="hs", name=f"h1sb{uid}m{m}")
                            for m in range(2)
                        ]
                        for m in range(2):
                            nc.scalar.activation(
                                h1sb[m][:],
                                h1ps[m][:],
                                Silu,
                                bias=bsb[:, 2 * e + m : 2 * e + m + 1],
                            )

                        # ---- layer 2: z2 = W2[e].T @ h1 (K=256 via 2 k-tiles) ----
                        h2ps = [
                            hps.tile([128, GW], F32, tag="h", name=f"h2ps{uid}m{m}")
                            for m in range(2)
                        ]
                        for m in range(2):
                            for k in range(2):
                                base = (2 * e + k) * H + m * 128
                                lhsT = w2sb[:, base : base + 128]
                                for t in range(GRP):
                                    ts = slice(t * BT, (t + 1) * BT)
                                    nc.tensor.matmul(
                                        h2ps[m][:, ts],
                                        lhsT,
                                        h1sb[k][:, ts],
                                        start=(k == 0),
                                        stop=(k == 1),
                                    )
                        h2sb = [
                            hpool.tile([128, GW], dt, tag="hs", name=f"h2sb{uid}m{m}")
                            for m in range(2)
                        ]
                        for m in range(2):
                            nc.scalar.activation(
                                h2sb[m][:],
                                h2ps[m][:],
                                Silu,
                                bias=bsb[:, 2 * E + 2 * e + m : 2 * E + 2 * e + m + 1],
                            )

                        # ---- layer 3: pred = W3[e].T @ h2 (K=256, M=48) ----
                        pps = ops.tile([OUT2, GW], F32, tag="o", name=f"pps{uid}")
                        for k in range(2):
                            base = (2 * e + k) * OUT2
                            lhsT = w3sb[:, base : base + OUT2]
                            for t in range(GRP):
                                ts = slice(t * BT, (t + 1) * BT)
                                nc.tensor.matmul(
                                    pps[:, ts],
                                    lhsT,
                                    h2sb[k][:, ts],
                                    start=(k == 0),
                                    stop=(k == 1),
                                )
                        # bias add on VectorE (PSUM -> SBUF fp32), then store
                        osb = opool.tile([OUT2, GW], F32, tag="os", name=f"osb{uid}")
                        nc.vector.tensor_scalar_add(
                            osb[:], pps[:], bsb[:OUT2, 4 * E + e : 4 * E + e + 1]
                        )
                        nc.sync.dma_start(out[e, :, c0 : c0 + GW], osb[:])

            if inner_reps == 1:
                body(0)
            else:
                # hardware loop for timing: rerun the whole computation
                # inner_reps times per launch (outputs just get overwritten)
                ET = mybir.EngineType
                with tc.For_i(
                    0,
                    inner_reps,
                    1,
                    hint_engines=(ET.PE, ET.Activation, ET.DVE, ET.SP, ET.Pool),
                ):
                    body(0)

    nc.finalize()
    return nc


def prep_inputs(x, W1, b1, W2, b2, W3, b3, compute="bf16"):
    """Host-side shard + repack into the per-core DRAM layouts."""
    npdt = ml_dtypes.bfloat16 if compute == "bf16" else np.float32
    x = np.asarray(x, np.float32)
    W1 = np.asarray(W1, np.float32)
    W2 = np.asarray(W2, np.float32)
    W3 = np.asarray(W3, np.float32)
    b1 = np.asarray(b1, np.float32)
    b2 = np.asarray(b2, np.float32)
    b3 = np.asarray(b3, np.float32)

    # weights: [K, M] stationary layouts, shared by all cores
    w1h = np.ascontiguousarray(W1.transpose(1, 0, 2).reshape(IN, E * H)).astype(npdt)
    w2h = np.ascontiguousarray(
        W2.reshape(E, 2, 128, H).transpose(2, 0, 1, 3).reshape(128, E * 2 * H)
    ).astype(npdt)
    w3h = np.ascontiguousarray(
        W3.reshape(E, 2, 128, OUT2).transpose(2, 0, 1, 3).reshape(128, E * 2 * OUT2)
    ).astype(npdt)

    # biases: [128, 5E] fp32; col 2e+m -> b1[e] m-half, col 2E+2e+m -> b2,
    # col 4E+e -> b3[e] in rows 0:48
    bsh = np.zeros((128, 5 * E), np.float32)
    for e in range(E):
        for m in range(2):
            bsh[:, 2 * e + m] = b1[e, m * 128 : (m + 1) * 128]
            bsh[:, 2 * E + 2 * e + m] = b2[e, m * 128 : (m + 1) * 128]
        bsh[:OUT2, 4 * E + e] = b3[e]

    in_maps = []
    xs = x.reshape(NCORES, BL, IN)
    for c in range(NCORES):
        xTc = np.ascontiguousarray(xs[c].T).astype(npdt)  # [IN, BL]
        in_maps.append(
            {"xT": xTc, "w1": w1h, "w2": w2h, "w3": w3h, "bs": bsh}
        )
    return in_maps


def assemble(results):
    """Per-core [E, 48, BL] fp32 -> (means, logvars) [E, B, 24] fp32."""
    full = np.concatenate([r["out"] for r in results], axis=2)  # [E, 48, B]
    pred = np.ascontiguousarray(full.transpose(0, 2, 1))        # [E, B, 48]
    means = np.ascontiguousarray(pred[:, :, :OUT])
    logvars = np.ascontiguousarray(pred[:, :, OUT:])
    return means, logvars


_NC_CACHE = {}


def _get_nc(compute, inner_reps=1):
    key = (compute, inner_reps)
    if key not in _NC_CACHE:
        _NC_CACHE[key] = build_nc(compute, inner_reps)
    return _NC_CACHE[key]


def kernel(x, W1, b1, W2, b2, W3, b3):
    compute = COMPUTE_DTYPE
    nc = _get_nc(compute)
    in_maps = prep_inputs(x, W1, b1, W2, b2, W3, b3, compute)
    res = run_bass_kernel_spmd(nc, in_maps, core_ids=list(range(NCORES)))
    return assemble(res.results)
